# revision 5
# baseline (speedup 1.0000x reference)
"""Trainium2 Bass kernel for the circular drift-diffusion loss (batched expm).

Reference computes  loss = -mean_b log(relu(e_{idx_b}^T expm(t_b*A) p0_b) + eps)
with A a fixed 360x360 circular advection-diffusion operator, t_b in [0,1000),
p0_b a von Mises density, over a batch of 256.

Algorithm (per core; batch sharded 32/core over 8 cores):
  * Quantize t_b = m_b*T0 + r_b with T0 = 1000/2^K, m_b < 2^K.
  * Build propagator chain M_j = expm(2^j*T0*A) once by repeated squaring
    (prelude: ascending Taylor at T0, then K-2 squarings).  A squaring is
    9 matmuls for S = M@M plus 9 PE transposes for S^T (the next stationary
    operand).  K and the Taylor degrees are chosen at runtime from ||A||_inf.
  * Apply bits of m_b as masked batched matvecs merged into the squaring
    matmuls: Q <- bit_j ? M_j Q : Q rides along as 32 extra moving columns.
  * Residual: Q <- Taylor_DEG_R(r_b A) Q (Horner, per-sample scalar folded
    into host-precomputed r/k coefficient tables).
  * p0 built on device (minimax poly cos + Exp activation), selection via
    one-hot + PE column-sum, loss terms via exponent/mantissa split + Ln.

Fast path (chosen when the resulting chain is short, k_bits <= 10): all
matrix/vector tiles in bf16 -> PE matmuls run 1 cycle/row (vs 4 for f32),
PE transposes 1 (vs 2), DVE elementwise gets 2-byte fast modes, DMA halves.
PSUM accumulation stays f32; p0's argument pipeline stays f32 (the von Mises
exponent needs f32 phase precision).  Empirically (numpy bit-sim) the bf16
chain keeps loss rel-err ~2e-4..4e-3 for plans with k_bits <= 10; for rare
heavy-diffusion inputs needing deeper chains the build falls back to f32.

Vector-engine work (PSUM evacuations, blends, Horner) is the secondary
bottleneck: the 9 per-level transpose evacuations are batched 3-into-1 via a
[120,360] PSUM staging bank, and all evacuations rotate across DVE/ACT/Pool.
"""

import math

import numpy as np

# ---------------- static problem constants (hardcoded per contract) ----------
N = 360            # color mesh size
P = 120            # partition chunk (N = 3*P)
NCH = 3            # chunks
B = 256            # total batch
NCORES = 8
BL = B // NCORES   # per-core batch
T_MAX = 1000.0
KAPPA = 400.0      # 1/SIGMA_INIT^2
EPS = 1e-5
TWO_PI = 6.283185307179586
# ln(1/(2*pi*i0e(400)))  [i0e(400) = 0.019953356281939987]
LNC = 2.076480848703078
# cos(sqrt(u)) on u in [0, pi^2] (|delta| folded to [0,pi]), minimax power
# basis c0..c5 (max err 1.75e-6 -> 7e-4 on log p0; tolerance is 2e-2)
COS_COEF = [0.9999982503105576, -0.4999925129381312, 0.0416590259231213,
            -0.0013857591185452258, 2.419643469550081e-05,
            -2.1969780329048054e-07]
# degree-8 Taylor-fit coefficients for the f32 fallback path (err 4e-14)
COS_COEF8 = [1.00000000e+00, -5.00000000e-01, 4.16666666e-02, -1.38888885e-03,
             2.48015646e-05, -2.75566515e-07, 2.08651966e-09, -1.13535474e-11,
             4.13131734e-14]

_COMPILED = {}

# fast-path dtype mode: "bf16" (2-byte tiles, PE 1cyc/row, DVE 2x modes) or
# "f32r" (f32 tiles, matmul operands viewed as float32r: PE 1cyc/row on wide
# matmuls at full f32 storage precision).  Chosen empirically on hardware.
FAST_MODE = "bf16"


def _taylor_deg(x, tol, lo):
    """Smallest d with x^(d+1)/(d+1)! < tol."""
    d = lo
    term = x ** (d + 1) / math.factorial(d + 1)
    while term > tol and d < 40:
        d += 1
        term *= x / (d + 1)
    return d


def _plan(anorm):
    """Choose (k_bits, deg_p, deg_r, fast) from ||A||_inf.  The time grid is
    T0 = T_MAX/2^k_bits; every squaring level applies one bit of the
    quantized delay.  fast=True selects the bf16 build."""
    xa = T_MAX * float(anorm)
    if xa <= 0.0:
        return 2, 4, 3, FAST_MODE

    def pick(c_lvl, c_pre, c_tay, tol_r):
        k0 = max(2, min(16, math.ceil(math.log2(max(xa / 0.9, 2.0)))))
        best = None
        for k in range(max(2, k0 - 1), min(16, k0 + 2) + 1):
            x0 = xa / (1 << k)
            tol_p = min(max(3e-5 / 2 ** (k / 2), 5e-8), 2e-6)
            dp = _taylor_deg(x0, tol_p, 4)
            dr = _taylor_deg(x0, tol_r, 3)
            cost = (k - 1) * c_lvl + (dp - 1) * c_pre + dr * c_tay
            if best is None or cost < best[0]:
                best = (cost, k, dp, dr)
        return best[1], best[2], best[3]

    fast_c = {"bf16": (2.5, 1.5, 0.35), "f32r": (2.7, 1.6, 0.55)}[FAST_MODE]
    k, dp, dr = pick(*fast_c, 2e-4)
    if k <= 10:
        return k, dp, dr, FAST_MODE
    k, dp, dr = pick(7.6, 4.5, 0.7, 1e-6)
    return k, dp, dr, "f32"


def _build_bass(k_bits, deg_p, deg_r, mode):
    """Construct the Bass program (SPMD; identical on all 8 cores)."""
    import concourse.tile as tile
    from concourse import bacc, mybir

    F32 = mybir.dt.float32
    R32 = mybir.dt.float32r
    BF = mybir.dt.bfloat16 if mode == "bf16" else F32
    MDT = BF if mode == "bf16" else mybir.dt.uint8   # mask dtype
    AF = mybir.ActivationFunctionType
    OP = mybir.AluOpType
    cos_coef = COS_COEF if mode != "f32" else COS_COEF8

    def mv(ap):
        # matmul operand view: f32r reinterpretation in f32r mode
        return ap.bitcast(R32) if mode == "f32r" else ap

    nc = bacc.Bacc("TRN2", target_bir_lowering=False, debug=False)

    def din(name, shape, dt=F32):
        return nc.dram_tensor(name, shape, dt, kind="ExternalInput").ap()

    d_x = din("x", [N, N], BF)    # X = T0*A
    d_xt = din("xt", [N, N], BF)  # X^T
    d_cm = din("cm", [P, NCH])    # c_mesh chunks: cm[p,c] = c_mesh[c*P+p]
    d_irep = din("irep", [P, BL])            # init_color replicated
    d_msk = din("msk", [P, k_bits * BL], MDT)  # bit masks (0/1)
    d_rdk = din("rdk", [P, deg_r * BL], BF)  # (r/T0)/k, k=1..deg_r
    d_oh = din("oh", [P, NCH * BL], BF)      # one-hot selector chunks
    d_out = nc.dram_tensor("terms", [1, BL], F32, kind="ExternalOutput").ap()

    with tile.TileContext(nc) as tc:
        with (
            tc.tile_pool(name="const", bufs=1) as cpool,
            tc.tile_pool(name="mats", bufs=3) as mpool,
            tc.tile_pool(name="qp", bufs=2) as qpool,
            tc.tile_pool(name="vp", bufs=3) as vpool,
            tc.tile_pool(name="tp", bufs=4) as tpool,
            tc.tile_pool(name="psb", bufs=3, space="PSUM") as psb,
            tc.tile_pool(name="pst", bufs=3, space="PSUM") as pstp,
            tc.tile_pool(name="pss", bufs=2, space="PSUM") as pss,
        ):
            # ---- constants ------------------------------------------------
            XN = cpool.tile([P, NCH * N], BF, tag="x")
            XT = cpool.tile([P, NCH * N], BF, tag="xt")
            # chunk 0 of x lands as [P,P] pieces plus the first XT block so
            # the first prelude matmuls start early; other chunks whole
            for b in range(NCH):
                nc.sync.dma_start(XN[:, b * P:(b + 1) * P],
                                  d_x[0:P, b * P:(b + 1) * P])
            nc.sync.dma_start(XT[:, 0:P], d_xt[0:P, 0:P])
            for c in range(1, NCH):
                nc.sync.dma_start(XN[:, c * N:(c + 1) * N],
                                  d_x[c * P:(c + 1) * P, :])
                nc.sync.dma_start(XT[:, c * N:(c + 1) * N],
                                  d_xt[c * P:(c + 1) * P, :])
            nc.sync.dma_start(XT[:, P:N], d_xt[0:P, P:N])
            CM = cpool.tile([P, NCH], F32, tag="cm")
            nc.sync.dma_start(CM[:], d_cm[:])
            IREP = cpool.tile([P, BL], F32, tag="irep")
            nc.sync.dma_start(IREP[:], d_irep[:])
            MSK = cpool.tile([P, k_bits * BL], MDT, tag="msk")
            nc.sync.dma_start(MSK[:], d_msk[:])
            RDK = cpool.tile([P, deg_r * BL], BF, tag="rdk")
            nc.sync.dma_start(RDK[:], d_rdk[:])
            OH = cpool.tile([P, NCH * BL], BF, tag="oh")
            nc.sync.dma_start(OH[:], d_oh[:])
            ONES = cpool.tile([P, 1], BF, tag="ones")
            nc.vector.memset(ONES[:], 1.0)
            BEXP = cpool.tile([P, 1], F32, tag="bexp")
            nc.vector.memset(BEXP[:], LNC - KAPPA)
            BLN0 = cpool.tile([1, 1], F32, tag="bln0")
            nc.vector.memset(BLN0[:], 0.0)
            # identities built on device (no DMA)
            EYE = cpool.tile([P, NCH * N], BF, tag="eye")
            nc.vector.memset(EYE[:], 1.0)
            nc.gpsimd.affine_select(
                EYE[:].rearrange("p (c n) -> p c n", c=NCH),
                EYE[:].rearrange("p (c n) -> p c n", c=NCH),
                pattern=[[-P, NCH], [1, N]], compare_op=OP.is_equal,
                fill=0.0, base=0, channel_multiplier=-1,
            )
            E120 = cpool.tile([P, P], BF, tag="e120")
            nc.vector.memset(E120[:], 1.0)
            nc.gpsimd.affine_select(
                E120[:], E120[:], pattern=[[1, P]], compare_op=OP.is_equal,
                fill=0.0, base=0, channel_multiplier=-1,
            )

            W = N + BL  # merged chunk width: [M_c | Q_c]

            # rotate PSUM->SBUF evacuations across DVE / ACT / Pool
            ev = {"i": 0}

            def evac(dst_ap, src_ap, scale=None):
                e = ev["i"] % 3
                ev["i"] += 1
                if scale is None:
                    if e == 0:
                        nc.vector.tensor_copy(dst_ap, src_ap)
                    elif e == 1:
                        nc.scalar.copy(dst_ap, src_ap)
                    else:
                        nc.gpsimd.tensor_copy(dst_ap, src_ap)
                else:
                    if e == 0:
                        nc.vector.tensor_scalar(dst_ap, src_ap, scale, None,
                                                op0=OP.mult)
                    elif e == 1:
                        nc.scalar.mul(dst_ap, src_ap, scale)
                    else:
                        nc.gpsimd.tensor_scalar(dst_ap, src_ap, scale, None,
                                                op0=OP.mult)

            def mm_group(ps, lhsT_tile, rhs_tile, i, rhs_w, rhs_stride=None):
                rs = rhs_w if rhs_stride is None else rhs_stride
                for c in range(NCH):
                    nc.tensor.matmul(
                        ps[:],
                        lhsT=mv(lhsT_tile[:, c * N + i * P: c * N + i * P + P]),
                        rhs=mv(rhs_tile[:, c * rs: c * rs + rhs_w]),
                        start=(c == 0), stop=(c == NCH - 1),
                    )

            # ---- p0 (von Mises) in Q-layout [P, NCH*BL] -------------------
            Q = qpool.tile([P, NCH * BL], BF, tag="q")
            deg_c = len(cos_coef) - 1
            for c in range(NCH):
                qs = Q[:, c * BL:(c + 1) * BL]
                dl = tpool.tile([P, BL], F32, tag="t0")
                # delta = init - c_mesh  (cos is even, sign irrelevant)
                nc.vector.tensor_scalar(dl[:], IREP[:], CM[:, c:c + 1], None,
                                        op0=OP.subtract)
                ab = tpool.tile([P, BL], F32, tag="t1")
                nc.scalar.activation(ab[:], dl[:], AF.Abs)
                fl = tpool.tile([P, BL], F32, tag="t2")
                nc.vector.tensor_scalar(fl[:], ab[:], -1.0, TWO_PI,
                                        op0=OP.mult, op1=OP.add)
                w = tpool.tile([P, BL], F32, tag="t3")
                nc.vector.tensor_tensor(w[:], ab[:], fl[:], op=OP.min)
                u = tpool.tile([P, BL], F32, tag="t0")
                nc.vector.tensor_tensor(u[:], w[:], w[:], op=OP.mult)
                h = tpool.tile([P, BL], F32, tag="t1")
                nc.vector.tensor_scalar(h[:], u[:], cos_coef[deg_c],
                                        cos_coef[deg_c - 1],
                                        op0=OP.mult, op1=OP.add)
                heng = nc.gpsimd if c == 1 else nc.vector
                for k in range(deg_c - 2, -1, -1):
                    heng.tensor_tensor(h[:], h[:], u[:], op=OP.mult)
                    heng.tensor_scalar(h[:], h[:], cos_coef[k], None,
                                       op0=OP.add)
                # p0 = exp(kappa*cos - kappa + lnC)
                nc.scalar.activation(qs, h[:], AF.Exp, bias=BEXP[:],
                                     scale=KAPPA)

            # ---- residual Taylor on p0 (commutes with the bit applies):
            # V = Q + rdk_k*(X V), k=deg_r..1.  Steps are emitted interleaved
            # with the prelude/chain so the small matmuls fill PE bubbles.
            taylor_state = {"V": Q, "k": deg_r, "dst": None}

            def taylor_step():
                k = taylor_state["k"]
                if k < 1:
                    return
                Vc = taylor_state["V"]
                last = k == 1 and taylor_state["dst"] is not None
                Vn = None if last else vpool.tile([P, NCH * BL], BF, tag="V")
                for i in range(NCH):
                    ps = pss.tile([P, BL], F32, tag="ap")
                    mm_group(ps, XT, Vc, i, BL)
                    if last:
                        # final step writes straight into the MQ tile's Q
                        # slots -- keeps the install copies off the
                        # prelude->chain critical path
                        vs = taylor_state["dst"][:, i * W + N: (i + 1) * W]
                    else:
                        vs = Vn[:, i * BL:(i + 1) * BL]
                    nc.vector.tensor_tensor(
                        vs, ps[:], RDK[:, (k - 1) * BL: k * BL], op=OP.mult)
                    nc.gpsimd.tensor_tensor(
                        vs, vs, Q[:, i * BL:(i + 1) * BL], op=OP.add)
                taylor_state["V"] = Vn
                taylor_state["k"] = k - 1

            # ---- prelude: ascending Taylor S = I + sum X^k/k! -------------
            # critical path per step: 9 matmuls + 3 scale-evacs; the S+=T
            # adds rotate across engines off the PE path.  M tiles are
            # MQ-shaped ([M_c | Q_c] per chunk) so later bit-applies merge
            # into the squaring matmuls as 32 extra moving columns.
            S = mpool.tile([P, NCH * W], BF, tag="M")
            taylor_state["dst"] = S
            for c in range(NCH):
                seng = (nc.vector, nc.gpsimd, nc.vector)[c % 3]
                seng.tensor_tensor(S[:, c * W: c * W + N],
                                   XN[:, c * N:(c + 1) * N],
                                   EYE[:, c * N:(c + 1) * N], op=OP.add)
            T = XN
            for k in range(2, deg_p + 1):
                Tn = mpool.tile([P, NCH * N], BF, tag="T")
                for i in range(NCH):
                    ps = psb.tile([P, N], F32, tag="sq")
                    mm_group(ps, XT, T, i, N)
                    evac(Tn[:, i * N:(i + 1) * N], ps[:], scale=1.0 / k)
                    seng = (nc.gpsimd, nc.vector, nc.gpsimd)[i % 3]
                    seng.tensor_tensor(S[:, i * W: i * W + N],
                                       S[:, i * W: i * W + N],
                                       Tn[:, i * N:(i + 1) * N],
                                       op=OP.add)
                T = Tn
                taylor_step()

            ST = mpool.tile([P, NCH * N], BF, tag="MT")

            def transpose_mq(MTt, Mt):
                # 3 PE transposes per MT chunk-row staged into one [P,N]
                # PSUM bank, then a single batched evacuation
                for cp in range(NCH):
                    pt = pstp.tile([P, N], BF, tag="tr")
                    for ib in range(NCH):
                        nc.tensor.transpose(
                            mv(pt[:, ib * P:(ib + 1) * P]),
                            mv(Mt[:, ib * W + cp * P: ib * W + cp * P + P]),
                            mv(E120[:]),
                        )
                    evac(MTt[:, cp * N:(cp + 1) * N], pt[:])

            transpose_mq(ST, S)
            M, MT = S, ST

            def square(Mc, MTc, bit=None):
                # Sn = Mc@Mc; if bit is not None also compute Mc@Q (merged
                # columns) and blend it into Sn's Q slot under the bit mask.
                Sn = mpool.tile([P, NCH * W], BF, tag="M")
                STn = mpool.tile([P, NCH * N], BF, tag="MT")
                wid = N if bit is None else W
                for i in range(NCH):
                    ps = psb.tile([P, wid], F32, tag="sq")
                    mm_group(ps, MTc, Mc, i, wid, rhs_stride=W)
                    evac(Sn[:, i * W: i * W + N], ps[:, :N])
                    if bit is not None:
                        nc.gpsimd.tensor_copy(Sn[:, i * W + N: (i + 1) * W],
                                              Mc[:, i * W + N: (i + 1) * W])
                        nc.vector.copy_predicated(
                            Sn[:, i * W + N: (i + 1) * W],
                            MSK[:, bit * BL:(bit + 1) * BL],
                            ps[:, N:W],
                        )
                transpose_mq(STn, Sn)
                return Sn, STn

            # drain remaining taylor steps; the k==1 step lands the
            # evolved p0 directly in the MQ tile's Q slots
            while taylor_state["k"] >= 1:
                taylor_step()

            # ---- merged bit applies + chain squarings ---------------------
            # level j squares M (= expm(2^j T0 A)) and applies bit j of the
            # quantized delay to Q in the same matmul set.  The top TWO bits
            # need no further squaring: bit k-2 is a single apply of M_{k-2}
            # and bit k-1 a double apply (M_{k-1} Q = M_{k-2} (M_{k-2} Q)),
            # which is ~2x cheaper than materializing M_{k-1}.
            for j in range(k_bits - 2):
                M, MT = square(M, MT, bit=j)

            def apply_bit(q_rhs, rhs_stride, rhs_off, bit, blend_src):
                # psum[i] = M @ q ; if bit is not None blend into a fresh
                # Q tile under the bit mask, else return psum tiles
                aps = []
                for i in range(NCH):
                    ps = pss.tile([P, BL], F32, tag="ap")
                    for c in range(NCH):
                        nc.tensor.matmul(
                            ps[:],
                            lhsT=mv(MT[:, c * N + i * P: c * N + i * P + P]),
                            rhs=mv(q_rhs[:, c * rhs_stride + rhs_off:
                                         c * rhs_stride + rhs_off + BL]),
                            start=(c == 0), stop=(c == NCH - 1),
                        )
                    aps.append(ps)
                if bit is None:
                    return aps
                Qn = qpool.tile([P, NCH * BL], BF, tag="qf")
                for i in range(NCH):
                    nc.gpsimd.tensor_copy(
                        Qn[:, i * BL:(i + 1) * BL],
                        blend_src[i] if isinstance(blend_src, list)
                        else blend_src[:, i * rhs_stride + rhs_off:
                                       i * rhs_stride + rhs_off + BL])
                    nc.vector.copy_predicated(
                        Qn[:, i * BL:(i + 1) * BL],
                        MSK[:, bit * BL:(bit + 1) * BL],
                        aps[i][:],
                    )
                return Qn

            # bit k-2: single apply on the Q slots of the MQ tile
            Qf = apply_bit(M, W, N, k_bits - 2, M)
            # bit k-1: double apply of the same M
            y1ps = apply_bit(Qf, BL, 0, None, None)
            Y1 = vpool.tile([P, NCH * BL], BF, tag="V")
            for i in range(NCH):
                evac(Y1[:, i * BL:(i + 1) * BL], y1ps[i][:])
            V = apply_bit(Y1, BL, 0, k_bits - 1, Qf)

            # ---- selection + loss terms -----------------------------------
            sel = psb.tile([1, BL], F32, tag="sq")
            for c in range(NCH):
                tmp = tpool.tile([P, BL], BF, tag="sel")
                nc.vector.tensor_tensor(
                    tmp[:], V[:, c * BL:(c + 1) * BL],
                    OH[:, c * BL:(c + 1) * BL], op=OP.mult)
                nc.tensor.matmul(sel[:], lhsT=mv(ONES[:]), rhs=mv(tmp[:]),
                                 start=(c == 0), stop=(c == NCH - 1))
            # ln(relu(psel)+eps) via exponent/mantissa split: the HW Ln
            # table degrades for huge args (psel can reach ~1e20 in the
            # weak-diffusion regime), so compute ln(m) + e*ln2 with m in
            # [1,2), which keeps the table in its accurate range.
            I32 = mybir.dt.int32
            rl = tpool.tile([1, BL], F32, tag="r0")
            nc.vector.tensor_scalar(rl[:], sel[:], 0.0, EPS,
                                    op0=OP.max, op1=OP.add)
            xi = rl[:].bitcast(I32)
            et = tpool.tile([1, BL], I32, tag="r2")
            nc.vector.tensor_scalar(et[:], xi, 23, None,
                                    op0=OP.arith_shift_right)
            ef = tpool.tile([1, BL], F32, tag="r3")
            nc.vector.tensor_copy(ef[:], et[:])
            mi = tpool.tile([1, BL], I32, tag="r4")
            nc.vector.tensor_scalar(mi[:], xi, 0x007FFFFF, 0x3F800000,
                                    op0=OP.bitwise_and, op1=OP.bitwise_or)
            lnm = tpool.tile([1, BL], F32, tag="r5")
            nc.scalar.activation(lnm[:], mi[:].bitcast(F32), AF.Ln,
                                 bias=BLN0[:], scale=1.0)
            terms = tpool.tile([1, BL], F32, tag="r1")
            # ef holds the biased exponent; fold the -127*ln2 into the mult
            nc.vector.tensor_scalar(terms[:], ef[:], 0.6931471805599453,
                                    -88.02969193111305,
                                    op0=OP.mult, op1=OP.add)
            nc.vector.tensor_tensor(terms[:], terms[:], lnm[:], op=OP.add)
            nc.sync.dma_start(d_out[:], terms[:])

    nc.compile()
    return nc


def _host_prep(c_mesh, gtheta, sigma_diff, init_color, delay_t, report_color):
    """Host-side glue: operator assembly (replicating reference f32 ops),
    plan selection, and per-core index/bit/layout arrays."""
    import ml_dtypes
    f32 = np.float32
    c = np.asarray(c_mesh, dtype=f32)
    g = np.asarray(gtheta, dtype=f32)
    s = np.asarray(sigma_diff, dtype=f32)[0]
    init = np.asarray(init_color, dtype=f32)
    t = np.asarray(delay_t, dtype=f32)
    rep = np.asarray(report_color, dtype=f32)

    d = (c[1] - c[0]).astype(f32)
    eye = np.eye(N, dtype=f32)
    up = np.roll(eye, -1, axis=1)
    dn = np.roll(eye, 1, axis=1)
    D1 = ((up - dn) / (f32(2.0) * d)).astype(f32)
    D2 = ((up - f32(2.0) * eye + dn) / (d * d)).astype(f32)
    A = ((s ** f32(2.0)) / f32(2.0) * D2 - D1 * g[None, :]).astype(f32)

    anorm = np.abs(A.astype(np.float64)).sum(axis=1).max()
    k_bits, deg_p, deg_r, mode = plan = _plan(anorm)
    bf = ml_dtypes.bfloat16 if mode == "bf16" else f32
    mdt = ml_dtypes.bfloat16 if mode == "bf16" else np.uint8
    T0 = T_MAX / (1 << k_bits)
    X = (A * f32(T0)).astype(f32)

    m = np.floor(t.astype(np.float64) / T0).astype(np.int64)
    m = np.clip(m, 0, (1 << k_bits) - 1)
    r = (t.astype(np.float64) - m * T0) / T0  # in X = T0*A units
    bits = ((m[:, None] >> np.arange(k_bits)[None, :]) & 1)     # [B, K]
    idx = np.argmin(np.abs(c[None, :] - rep[:, None]), axis=1)

    shared = {
        "x": X.astype(bf),
        "xt": np.ascontiguousarray(X.T).astype(bf),
        "cm": np.ascontiguousarray(c.reshape(NCH, P).T),
    }
    in_maps = []
    for core in range(NCORES):
        sl = slice(core * BL, (core + 1) * BL)
        irep = np.broadcast_to(init[sl][None, :], (P, BL)).astype(f32)
        msk = np.broadcast_to(
            bits[sl].T.reshape(1, k_bits * BL), (P, k_bits * BL)
        ).astype(mdt)  # bit j at [j*BL:(j+1)*BL]
        rdk = np.empty((deg_r, BL), f32)
        for k in range(1, deg_r + 1):
            rdk[k - 1] = (r[sl] / k).astype(f32)
        rdk = np.broadcast_to(
            rdk.reshape(1, deg_r * BL), (P, deg_r * BL)).astype(bf)
        oh = np.zeros((NCH, P, BL), f32)
        for b, ix in enumerate(idx[sl]):
            oh[ix // P, ix % P, b] = 1.0
        oh = np.ascontiguousarray(
            oh.transpose(1, 0, 2).reshape(P, NCH * BL)).astype(bf)
        in_maps.append(dict(shared, irep=irep, msk=msk, rdk=rdk, oh=oh))
    return plan, in_maps


def _get_nc(plan):
    if plan not in _COMPILED:
        _COMPILED[plan] = _build_bass(*plan)
    return _COMPILED[plan]


def kernel(**inputs):
    from concourse.bass_utils import run_bass_kernel_spmd

    plan, in_maps = _host_prep(
        inputs["c_mesh"], inputs["gtheta"], inputs["sigma_diff"],
        inputs["init_color"], inputs["delay_t"], inputs["report_color"],
    )
    nc = _get_nc(plan)
    res = run_bass_kernel_spmd(nc, in_maps, list(range(NCORES)))
    terms = np.concatenate(
        [np.asarray(res.results[k]["terms"]).reshape(-1) for k in range(NCORES)]
    )
    loss = -np.mean(terms.astype(np.float64))
    return np.asarray(loss, dtype=np.float32)


# revision 8
# speedup vs baseline: 1.1170x; 1.1170x over previous
"""Trainium2 Bass kernel for the circular drift-diffusion loss (batched expm).

Reference computes  loss = -mean_b log(relu(e_{idx_b}^T expm(t_b*A) p0_b) + eps)
with A a fixed 360x360 circular advection-diffusion operator, t_b in [0,1000),
p0_b a von Mises density, over a batch of 256.

Algorithm (per core; batch sharded 32/core over 8 cores):
  * Quantize t_b = m_b*T0 + r_b with T0 = 1000/2^K, m_b < 2^K.
  * M-chain: build M_j = expm(2^j*T0*A) by repeated squaring (prelude:
    ascending Taylor at T0, then K-2 squarings).  A squaring is 9 wide
    matmuls for S = M@M plus 9 PE transposes for S^T (the next stationary
    operand); the last squaring computes S^T directly (matmul with swapped
    operands) since its row form is never consumed.
  * Q-chain (decoupled, runs one level behind the M-chain off the critical
    path): p0 -> residual Taylor_DEG_R(r_b A) (Horner with host-precomputed
    r/k tables) -> per-level masked batched matvecs Q <- bit_j ? M_j Q : Q.
    The top TWO bits reuse M_{K-2}: bit K-2 one apply, bit K-1 two applies.
  * p0 built on device (minimax poly cos + Exp activation), selection via
    one-hot + PE column-sum, loss terms via exponent/mantissa split + Ln.

Fast path (chosen when the chain is short, k_bits <= 10): all matrix/vector
tiles bf16 -> PE matmuls 1 cycle/row (vs 4 for f32), transposes 1 (vs 2),
DVE elementwise gets 2-byte fast modes, DMA halves.  PSUM stays f32; p0's
phase pipeline stays f32.  Numpy bit-simulation keeps loss rel-err
2e-4..9e-3 for k_bits <= 10 plans (gate 2e-2); deeper chains fall back to
f32.  "f32r" mode (f32 tiles, matmul operands viewed as float32r) is the
full-precision alternative at the same wide-matmul rate.

Latency structure: inputs arrive on two HWDGE queues (SP + ACT) in
consumption order; per squaring the three S-row evacuations are staggered
behind their PSUM sub-groups, MT row 0 is evacuated in [P,120] pieces so the
next level's first matmuls unblock early, and rows 1/2 evacuate batched.
"""

import math

import numpy as np

# ---------------- static problem constants (hardcoded per contract) ----------
N = 360            # color mesh size
P = 120            # partition chunk (N = 3*P)
NCH = 3            # chunks
B = 256            # total batch
NCORES = 8
BL = B // NCORES   # per-core batch
QW = NCH * BL      # Q-chain tile width (96)
T_MAX = 1000.0
KAPPA = 400.0      # 1/SIGMA_INIT^2
EPS = 1e-5
TWO_PI = 6.283185307179586
# ln(1/(2*pi*i0e(400)))  [i0e(400) = 0.019953356281939987]
LNC = 2.076480848703078
# cos(sqrt(u)) on u in [0, pi^2] (|delta| folded to [0,pi]), minimax power
# basis c0..c5 (max err 1.75e-6 -> 7e-4 on log p0; tolerance is 2e-2)
COS_COEF = [0.9999982503105576, -0.4999925129381312, 0.0416590259231213,
            -0.0013857591185452258, 2.419643469550081e-05,
            -2.1969780329048054e-07]
# degree-8 Taylor-fit coefficients for the f32 fallback path (err 4e-14)
COS_COEF8 = [1.00000000e+00, -5.00000000e-01, 4.16666666e-02, -1.38888885e-03,
             2.48015646e-05, -2.75566515e-07, 2.08651966e-09, -1.13535474e-11,
             4.13131734e-14]

_COMPILED = {}

# fast-path dtype mode: "bf16" (2-byte tiles, PE 1cyc/row, DVE 2x modes) or
# "f32r" (f32 tiles, matmul operands viewed as float32r: PE 1cyc/row on wide
# matmuls at full f32 storage precision).  Chosen empirically on hardware.
FAST_MODE = "bf16"


def _taylor_deg(x, tol, lo):
    """Smallest d with x^(d+1)/(d+1)! < tol."""
    d = lo
    term = x ** (d + 1) / math.factorial(d + 1)
    while term > tol and d < 40:
        d += 1
        term *= x / (d + 1)
    return d


def _plan(anorm):
    """Choose (k_bits, deg_p, deg_r, mode) from ||A||_inf.  The time grid is
    T0 = T_MAX/2^k_bits; every squaring level applies one bit of the
    quantized delay."""
    xa = T_MAX * float(anorm)
    if xa <= 0.0:
        return 3, 4, 3, FAST_MODE

    def pick(c_lvl, c_pre, c_tay, tol_r, tol_p_num):
        k0 = max(3, min(16, math.ceil(math.log2(max(xa / 0.9, 2.0)))))
        best = None
        for k in range(max(3, k0 - 1), min(16, k0 + 2) + 1):
            x0 = xa / (1 << k)
            tol_p = min(max(tol_p_num / 2 ** (k / 2), 5e-8), 2e-5)
            dp = _taylor_deg(x0, tol_p, 4)
            dr = _taylor_deg(x0, tol_r, 3)
            cost = (k - 1) * c_lvl + (dp - 1) * c_pre + dr * c_tay
            if best is None or cost < best[0]:
                best = (cost, k, dp, dr)
        return best[1], best[2], best[3]

    fast_c = {"bf16": (2.2, 1.6, 0.6), "f32r": (2.5, 1.7, 0.7)}[FAST_MODE]
    k, dp, dr = pick(*fast_c, 2e-4, 3e-4)
    if k <= 10:
        return k, dp, dr, FAST_MODE
    k, dp, dr = pick(7.6, 4.5, 0.7, 1e-6, 3e-5)
    return k, dp, dr, "f32"


def _build_bass(k_bits, deg_p, deg_r, mode):
    """Construct the Bass program (SPMD; identical on all 8 cores)."""
    import concourse.tile as tile
    from concourse import bacc, mybir

    F32 = mybir.dt.float32
    R32 = mybir.dt.float32r
    BF = mybir.dt.bfloat16 if mode == "bf16" else F32
    MDT = BF if mode == "bf16" else mybir.dt.uint8   # mask dtype
    AF = mybir.ActivationFunctionType
    OP = mybir.AluOpType
    cos_coef = COS_COEF if mode != "f32" else COS_COEF8

    def mv(ap):
        # matmul operand view: f32r reinterpretation in f32r mode
        return ap.bitcast(R32) if mode == "f32r" else ap

    nc = bacc.Bacc("TRN2", target_bir_lowering=False, debug=False)

    def din(name, shape, dt=F32):
        return nc.dram_tensor(name, shape, dt, kind="ExternalInput").ap()

    d_x = din("x", [N, N], BF)    # X = T0*A
    d_xt = din("xt", [N, N], BF)  # X^T
    d_cm = din("cm", [P, NCH])    # c_mesh chunks: cm[p,c] = c_mesh[c*P+p]
    d_irep = din("irep", [P, BL])            # init_color replicated
    d_msk = din("msk", [P, k_bits * QW], MDT)  # bit masks (0/1), x3 chunks
    d_rdk = din("rdk", [P, deg_r * QW], BF)  # (r/T0)/k x3 chunks, k=1..deg_r
    d_oh = din("oh", [P, QW], BF)            # one-hot selector chunks
    d_out = nc.dram_tensor("terms", [1, BL], F32, kind="ExternalOutput").ap()

    with tile.TileContext(nc) as tc:
        with (
            tc.tile_pool(name="const", bufs=1) as cpool,
            tc.tile_pool(name="mats", bufs=3) as mpool,
            tc.tile_pool(name="qp", bufs=3) as qpool,
            tc.tile_pool(name="tp", bufs=4) as tpool,
            tc.tile_pool(name="psb", bufs=3, space="PSUM") as psb,
            tc.tile_pool(name="pst", bufs=3, space="PSUM") as pstp,
            tc.tile_pool(name="pss", bufs=2, space="PSUM") as pss,
        ):
            # ---- input DMAs on two HWDGE queues, consumption order --------
            XN = cpool.tile([P, NCH * N], BF, tag="x")
            XT = cpool.tile([P, NCH * N], BF, tag="xt")
            CM = cpool.tile([P, NCH], F32, tag="cm")
            IREP = cpool.tile([P, BL], F32, tag="irep")
            MSK = cpool.tile([P, k_bits * QW], MDT, tag="msk")
            RDK = cpool.tile([P, deg_r * QW], BF, tag="rdk")
            OH = cpool.tile([P, QW], BF, tag="oh")
            # SP queue: matmul operands, first-needed first (chunk-0 pieces
            # let the first prelude matmuls start before the bulk lands)
            nc.sync.dma_start(XT[:, 0:P], d_xt[0:P, 0:P])
            for b in range(NCH):
                nc.sync.dma_start(XN[:, b * P:(b + 1) * P],
                                  d_x[0:P, b * P:(b + 1) * P])
            for c in range(1, NCH):
                nc.sync.dma_start(XT[:, c * N:(c + 1) * N],
                                  d_xt[c * P:(c + 1) * P, :])
                nc.sync.dma_start(XN[:, c * N:(c + 1) * N],
                                  d_x[c * P:(c + 1) * P, :])
            nc.sync.dma_start(XT[:, P:N], d_xt[0:P, P:N])
            # ACT queue: p0 inputs first, then Q-chain tables
            nc.scalar.dma_start(CM[:], d_cm[:])
            nc.scalar.dma_start(IREP[:], d_irep[:])
            nc.scalar.dma_start(RDK[:], d_rdk[:])
            nc.scalar.dma_start(MSK[:], d_msk[:])
            nc.scalar.dma_start(OH[:], d_oh[:])

            ONES = cpool.tile([P, 1], BF, tag="ones")
            nc.vector.memset(ONES[:], 1.0)
            BEXP = cpool.tile([P, 1], F32, tag="bexp")
            nc.vector.memset(BEXP[:], LNC - KAPPA)
            BLN0 = cpool.tile([1, 1], F32, tag="bln0")
            nc.vector.memset(BLN0[:], 0.0)
            # identities built on device (no DMA)
            EYE = cpool.tile([P, NCH * N], BF, tag="eye")
            nc.vector.memset(EYE[:], 1.0)
            nc.gpsimd.affine_select(
                EYE[:].rearrange("p (c n) -> p c n", c=NCH),
                EYE[:].rearrange("p (c n) -> p c n", c=NCH),
                pattern=[[-P, NCH], [1, N]], compare_op=OP.is_equal,
                fill=0.0, base=0, channel_multiplier=-1,
            )
            E120 = cpool.tile([P, P], BF, tag="e120")
            nc.vector.memset(E120[:], 1.0)
            nc.gpsimd.affine_select(
                E120[:], E120[:], pattern=[[1, P]], compare_op=OP.is_equal,
                fill=0.0, base=0, channel_multiplier=-1,
            )

            def mm_group(ps, lhsT_tile, rhs_tile, i, rhs_w):
                for c in range(NCH):
                    nc.tensor.matmul(
                        ps[:],
                        lhsT=mv(lhsT_tile[:, c * N + i * P: c * N + i * P + P]),
                        rhs=mv(rhs_tile[:, c * rhs_w: (c + 1) * rhs_w]),
                        start=(c == 0), stop=(c == NCH - 1),
                    )

            # ---- p0 (von Mises) in Q-layout [P, 3*BL] ---------------------
            Q0 = qpool.tile([P, QW], BF, tag="q")
            deg_c = len(cos_coef) - 1
            for c in range(NCH):
                qs = Q0[:, c * BL:(c + 1) * BL]
                dl = tpool.tile([P, BL], F32, tag="t0")
                # delta = init - c_mesh  (cos is even, sign irrelevant)
                nc.vector.tensor_scalar(dl[:], IREP[:], CM[:, c:c + 1], None,
                                        op0=OP.subtract)
                ab = tpool.tile([P, BL], F32, tag="t1")
                nc.scalar.activation(ab[:], dl[:], AF.Abs)
                fl = tpool.tile([P, BL], F32, tag="t2")
                nc.vector.tensor_scalar(fl[:], ab[:], -1.0, TWO_PI,
                                        op0=OP.mult, op1=OP.add)
                w = tpool.tile([P, BL], F32, tag="t3")
                nc.vector.tensor_tensor(w[:], ab[:], fl[:], op=OP.min)
                u = tpool.tile([P, BL], F32, tag="t0")
                nc.vector.tensor_tensor(u[:], w[:], w[:], op=OP.mult)
                h = tpool.tile([P, BL], F32, tag="t1")
                nc.vector.tensor_scalar(h[:], u[:], cos_coef[deg_c],
                                        cos_coef[deg_c - 1],
                                        op0=OP.mult, op1=OP.add)
                heng = nc.gpsimd if c == 1 else nc.vector
                for k in range(deg_c - 2, -1, -1):
                    heng.tensor_tensor(h[:], h[:], u[:], op=OP.mult)
                    heng.tensor_scalar(h[:], h[:], cos_coef[k], None,
                                       op0=OP.add)
                # p0 = exp(kappa*cos - kappa + lnC)
                nc.scalar.activation(qs, h[:], AF.Exp, bias=BEXP[:],
                                     scale=KAPPA)

            # ---- Q-chain step emitters (off the M-chain critical path) ----
            # residual Taylor: V <- Q0 + rdk_k*(X V), k=deg_r..1
            taylor_state = {"V": Q0, "k": deg_r}

            def taylor_step():
                k = taylor_state["k"]
                if k < 1:
                    return
                Vc = taylor_state["V"]
                ps = pss.tile([P, QW], F32, tag="ap")
                for i in range(NCH):
                    for c in range(NCH):
                        nc.tensor.matmul(
                            ps[:, i * BL:(i + 1) * BL],
                            lhsT=mv(XT[:, c * N + i * P: c * N + i * P + P]),
                            rhs=mv(Vc[:, c * BL:(c + 1) * BL]),
                            start=(c == 0), stop=(c == NCH - 1),
                        )
                Vn = qpool.tile([P, QW], BF, tag="v")
                nc.vector.tensor_tensor(Vn[:], ps[:],
                                        RDK[:, (k - 1) * QW: k * QW],
                                        op=OP.mult)
                nc.gpsimd.tensor_tensor(Vn[:], Vn[:], Q0[:], op=OP.add)
                taylor_state["V"] = Vn
                taylor_state["k"] = k - 1

            def apply_bit(MTj, q_tile, bit, blend_src=None):
                # Qn = bit ? M_j @ q : blend_src   (9 narrow mms + one blend)
                ps = pss.tile([P, QW], F32, tag="ap")
                for i in range(NCH):
                    for c in range(NCH):
                        nc.tensor.matmul(
                            ps[:, i * BL:(i + 1) * BL],
                            lhsT=mv(MTj[:, c * N + i * P: c * N + i * P + P]),
                            rhs=mv(q_tile[:, c * BL:(c + 1) * BL]),
                            start=(c == 0), stop=(c == NCH - 1),
                        )
                Qn = qpool.tile([P, QW], BF, tag="q")
                nc.gpsimd.tensor_copy(
                    Qn[:], (q_tile if blend_src is None else blend_src)[:])
                nc.vector.copy_predicated(
                    Qn[:], MSK[:, bit * QW:(bit + 1) * QW], ps[:])
                return Qn

            # ---- prelude: ascending Taylor S = I + sum X^k/k! -------------
            S = mpool.tile([P, NCH * N], BF, tag="M")
            for c in range(NCH):
                nc.vector.tensor_tensor(S[:, c * N:(c + 1) * N],
                                        XN[:, c * N:(c + 1) * N],
                                        EYE[:, c * N:(c + 1) * N], op=OP.add)
            T = XN
            for k in range(2, deg_p + 1):
                Tn = mpool.tile([P, NCH * N], BF, tag="T")
                for i in range(NCH):
                    ps = psb.tile([P, N], F32, tag="sq")
                    mm_group(ps, XT, T, i, N)
                    # scale-evac (ACT) feeds the next step's matmuls; the
                    # S accumulation runs on DVE off the PE path
                    nc.scalar.mul(Tn[:, i * N:(i + 1) * N], ps[:], 1.0 / k)
                    nc.vector.tensor_tensor(S[:, i * N:(i + 1) * N],
                                            S[:, i * N:(i + 1) * N],
                                            Tn[:, i * N:(i + 1) * N],
                                            op=OP.add)
                T = Tn
                taylor_step()
            while taylor_state["k"] >= 1:   # drain (off critical path)
                taylor_step()
            QB = taylor_state["V"]

            def transpose_mq(MTt, Mt):
                # MT row 0 evacuates in [P,P] pieces (piece i unblocks the
                # next level's c=0 matmuls for output chunk i); rows 1-2
                # evacuate batched from a shared PSUM staging bank.
                for cp in range(NCH):
                    pt = pstp.tile([P, N], BF, tag="tr")
                    for ib in range(NCH):
                        nc.tensor.transpose(
                            mv(pt[:, ib * P:(ib + 1) * P]),
                            mv(Mt[:, ib * N + cp * P: ib * N + cp * P + P]),
                            mv(E120[:]),
                        )
                        if cp == 0:
                            eng = (nc.vector, nc.scalar, nc.vector)[ib]
                            dst = MTt[:, ib * P:(ib + 1) * P]
                            if eng is nc.scalar:
                                nc.scalar.copy(dst, pt[:, ib * P:(ib + 1) * P])
                            else:
                                nc.vector.tensor_copy(
                                    dst, pt[:, ib * P:(ib + 1) * P])
                    if cp == 1:
                        nc.scalar.copy(MTt[:, cp * N:(cp + 1) * N], pt[:])
                    elif cp == 2:
                        nc.vector.tensor_copy(MTt[:, cp * N:(cp + 1) * N],
                                              pt[:])

            MT = mpool.tile([P, NCH * N], BF, tag="MT")
            transpose_mq(MT, S)

            # ---- chain: squarings with lag-1 merged bit applies -----------
            # squaring s: M_s = M_{s-1}^2 (wide mms + transposes); bit s-1
            # applied right after the wide mms (its operands are a full level
            # old, so the PE never stalls on the Q-chain).
            n_sq = k_bits - 2
            for s in range(1, n_sq + 1):
                last = s == n_sq
                if not last:
                    Sn = mpool.tile([P, NCH * N], BF, tag="M")
                    for i in range(NCH):
                        ps = psb.tile([P, N], F32, tag="sq")
                        mm_group(ps, MT, S, i, N)
                        eng = (nc.scalar, nc.vector, nc.scalar)[i]
                        if eng is nc.scalar:
                            nc.scalar.copy(Sn[:, i * N:(i + 1) * N], ps[:])
                        else:
                            nc.vector.tensor_copy(Sn[:, i * N:(i + 1) * N],
                                                  ps[:])
                else:
                    # last squaring: only M_{n_sq}^T is ever consumed (by the
                    # top-bit applies) -> compute S^T = M^T@M^T directly and
                    # skip the row form + transposes entirely
                    MTn = mpool.tile([P, NCH * N], BF, tag="MT")
                    for i in range(NCH):
                        ps = psb.tile([P, N], F32, tag="sq")
                        for c in range(NCH):
                            nc.tensor.matmul(
                                ps[:],
                                lhsT=mv(S[:, c * N + i * P:
                                          c * N + i * P + P]),
                                rhs=mv(MT[:, c * N: (c + 1) * N]),
                                start=(c == 0), stop=(c == NCH - 1),
                            )
                        eng = (nc.scalar, nc.vector, nc.scalar)[i]
                        if eng is nc.scalar:
                            nc.scalar.copy(MTn[:, i * N:(i + 1) * N], ps[:])
                        else:
                            nc.vector.tensor_copy(MTn[:, i * N:(i + 1) * N],
                                                  ps[:])
                # bit s-1 rides on M_{s-1} (= current MT) while evacs run
                QB = apply_bit(MT, QB, s - 1)
                if not last:
                    MTn = mpool.tile([P, NCH * N], BF, tag="MT")
                    transpose_mq(MTn, Sn)
                    S = Sn
                MT = MTn

            # ---- top two bits: single + double apply of M_{k-2} -----------
            QB = apply_bit(MT, QB, k_bits - 2)
            psy = pss.tile([P, QW], F32, tag="ap")
            for i in range(NCH):
                for c in range(NCH):
                    nc.tensor.matmul(
                        psy[:, i * BL:(i + 1) * BL],
                        lhsT=mv(MT[:, c * N + i * P: c * N + i * P + P]),
                        rhs=mv(QB[:, c * BL:(c + 1) * BL]),
                        start=(c == 0), stop=(c == NCH - 1),
                    )
            Y1 = qpool.tile([P, QW], BF, tag="v")
            nc.vector.tensor_copy(Y1[:], psy[:])
            Vf = apply_bit(MT, Y1, k_bits - 1, blend_src=QB)

            # ---- selection + loss terms -----------------------------------
            sel = psb.tile([1, BL], F32, tag="sq")
            for c in range(NCH):
                tmp = tpool.tile([P, BL], BF, tag="sel")
                nc.vector.tensor_tensor(
                    tmp[:], Vf[:, c * BL:(c + 1) * BL],
                    OH[:, c * BL:(c + 1) * BL], op=OP.mult)
                nc.tensor.matmul(sel[:], lhsT=mv(ONES[:]), rhs=mv(tmp[:]),
                                 start=(c == 0), stop=(c == NCH - 1))
            # ln(relu(psel)+eps) via exponent/mantissa split: the HW Ln
            # table degrades for huge args (psel can reach ~1e20 in the
            # weak-diffusion regime), so compute ln(m) + e*ln2 with m in
            # [1,2), which keeps the table in its accurate range.
            I32 = mybir.dt.int32
            rl = tpool.tile([1, BL], F32, tag="r0")
            nc.vector.tensor_scalar(rl[:], sel[:], 0.0, EPS,
                                    op0=OP.max, op1=OP.add)
            xi = rl[:].bitcast(I32)
            et = tpool.tile([1, BL], I32, tag="r2")
            nc.vector.tensor_scalar(et[:], xi, 23, None,
                                    op0=OP.arith_shift_right)
            ef = tpool.tile([1, BL], F32, tag="r3")
            nc.vector.tensor_copy(ef[:], et[:])
            mi = tpool.tile([1, BL], I32, tag="r4")
            nc.vector.tensor_scalar(mi[:], xi, 0x007FFFFF, 0x3F800000,
                                    op0=OP.bitwise_and, op1=OP.bitwise_or)
            lnm = tpool.tile([1, BL], F32, tag="r5")
            nc.scalar.activation(lnm[:], mi[:].bitcast(F32), AF.Ln,
                                 bias=BLN0[:], scale=1.0)
            terms = tpool.tile([1, BL], F32, tag="r1")
            # ef holds the biased exponent; fold the -127*ln2 into the mult
            nc.vector.tensor_scalar(terms[:], ef[:], 0.6931471805599453,
                                    -88.02969193111305,
                                    op0=OP.mult, op1=OP.add)
            nc.vector.tensor_tensor(terms[:], terms[:], lnm[:], op=OP.add)
            nc.sync.dma_start(d_out[:], terms[:])

    nc.compile()
    return nc


def _host_prep(c_mesh, gtheta, sigma_diff, init_color, delay_t, report_color):
    """Host-side glue: operator assembly (replicating reference f32 ops),
    plan selection, and per-core index/bit/layout arrays."""
    import ml_dtypes
    f32 = np.float32
    c = np.asarray(c_mesh, dtype=f32)
    g = np.asarray(gtheta, dtype=f32)
    s = np.asarray(sigma_diff, dtype=f32)[0]
    init = np.asarray(init_color, dtype=f32)
    t = np.asarray(delay_t, dtype=f32)
    rep = np.asarray(report_color, dtype=f32)

    d = (c[1] - c[0]).astype(f32)
    eye = np.eye(N, dtype=f32)
    up = np.roll(eye, -1, axis=1)
    dn = np.roll(eye, 1, axis=1)
    D1 = ((up - dn) / (f32(2.0) * d)).astype(f32)
    D2 = ((up - f32(2.0) * eye + dn) / (d * d)).astype(f32)
    A = ((s ** f32(2.0)) / f32(2.0) * D2 - D1 * g[None, :]).astype(f32)

    anorm = np.abs(A.astype(np.float64)).sum(axis=1).max()
    k_bits, deg_p, deg_r, mode = plan = _plan(anorm)
    bf = ml_dtypes.bfloat16 if mode == "bf16" else f32
    mdt = ml_dtypes.bfloat16 if mode == "bf16" else np.uint8
    T0 = T_MAX / (1 << k_bits)
    X = (A * f32(T0)).astype(f32)

    m = np.floor(t.astype(np.float64) / T0).astype(np.int64)
    m = np.clip(m, 0, (1 << k_bits) - 1)
    r = (t.astype(np.float64) - m * T0) / T0  # in X = T0*A units
    bits = ((m[:, None] >> np.arange(k_bits)[None, :]) & 1)     # [B, K]
    idx = np.argmin(np.abs(c[None, :] - rep[:, None]), axis=1)

    shared = {
        "x": X.astype(bf),
        "xt": np.ascontiguousarray(X.T).astype(bf),
        "cm": np.ascontiguousarray(c.reshape(NCH, P).T),
    }
    in_maps = []
    for core in range(NCORES):
        sl = slice(core * BL, (core + 1) * BL)
        irep = np.broadcast_to(init[sl][None, :], (P, BL)).astype(f32)
        # bit j replicated x3 (one copy per Q chunk) at [j*QW:(j+1)*QW]
        msk = np.broadcast_to(
            np.tile(bits[sl].T.reshape(k_bits, 1, BL), (1, NCH, 1))
            .reshape(1, k_bits * QW), (P, k_bits * QW)).astype(mdt)
        rdk = np.empty((deg_r, BL), f32)
        for k in range(1, deg_r + 1):
            rdk[k - 1] = (r[sl] / k).astype(f32)
        rdk = np.broadcast_to(
            np.tile(rdk.reshape(deg_r, 1, BL), (1, NCH, 1))
            .reshape(1, deg_r * QW), (P, deg_r * QW)).astype(bf)
        oh = np.zeros((NCH, P, BL), f32)
        for b, ix in enumerate(idx[sl]):
            oh[ix // P, ix % P, b] = 1.0
        oh = np.ascontiguousarray(
            oh.transpose(1, 0, 2).reshape(P, QW)).astype(bf)
        in_maps.append(dict(shared, irep=irep, msk=msk, rdk=rdk, oh=oh))
    return plan, in_maps


def _get_nc(plan):
    if plan not in _COMPILED:
        _COMPILED[plan] = _build_bass(*plan)
    return _COMPILED[plan]


def kernel(**inputs):
    from concourse.bass_utils import run_bass_kernel_spmd

    plan, in_maps = _host_prep(
        inputs["c_mesh"], inputs["gtheta"], inputs["sigma_diff"],
        inputs["init_color"], inputs["delay_t"], inputs["report_color"],
    )
    nc = _get_nc(plan)
    res = run_bass_kernel_spmd(nc, in_maps, list(range(NCORES)))
    terms = np.concatenate(
        [np.asarray(res.results[k]["terms"]).reshape(-1) for k in range(NCORES)]
    )
    loss = -np.mean(terms.astype(np.float64))
    return np.asarray(loss, dtype=np.float32)


# revision 10
# speedup vs baseline: 1.2396x; 1.1098x over previous
"""Trainium2 Bass kernel for the circular drift-diffusion loss (batched expm).

Reference computes  loss = -mean_b log(relu(e_{idx_b}^T expm(t_b*A) p0_b) + eps)
with A a fixed 360x360 circular advection-diffusion operator, t_b in [0,1000),
p0_b a von Mises density, over a batch of 256.

Algorithm (per core; batch sharded 32/core over 8 cores):
  * Quantize t_b = m_b*T0 + r_b with T0 = 1000/2^K, m_b < 2^K.
  * M-chain: build M_j = expm(2^j*T0*A) by repeated squaring (prelude:
    ascending Taylor at T0, then K-2 squarings).  A squaring is 9 wide
    matmuls for S = M@M plus 9 PE transposes for S^T (the next stationary
    operand); the last squaring computes S^T directly (matmul with swapped
    operands) since its row form is never consumed.
  * Q-chain (decoupled, runs one level behind the M-chain off the critical
    path): p0 -> residual Taylor_DEG_R(r_b A) (Horner with host-precomputed
    r/k tables) -> per-level masked batched matvecs Q <- bit_j ? M_j Q : Q.
    The top TWO bits reuse M_{K-2}: bit K-2 one apply, bit K-1 two applies.
  * p0 built on device (minimax poly cos + Exp activation), selection via
    one-hot + PE column-sum, loss terms via exponent/mantissa split + Ln.

Fast path (chosen when the chain is short, k_bits <= 10): all matrix/vector
tiles bf16 -> PE matmuls 1 cycle/row (vs 4 for f32), transposes 1 (vs 2),
DVE elementwise gets 2-byte fast modes, DMA halves.  PSUM stays f32; p0's
phase pipeline stays f32.  Numpy bit-simulation keeps loss rel-err
2e-4..9e-3 for k_bits <= 10 plans (gate 2e-2); deeper chains fall back to
f32.  "f32r" mode (f32 tiles, matmul operands viewed as float32r) is the
full-precision alternative at the same wide-matmul rate.

Latency structure: inputs arrive on two HWDGE queues (SP + ACT) in
consumption order; per squaring the three S-row evacuations are staggered
behind their PSUM sub-groups, MT row 0 is evacuated in [P,120] pieces so the
next level's first matmuls unblock early, and rows 1/2 evacuate batched.
"""

import math

import numpy as np

# ---------------- static problem constants (hardcoded per contract) ----------
N = 360            # color mesh size
P = 120            # partition chunk (N = 3*P)
NCH = 3            # chunks
B = 256            # total batch
NCORES = 8
BL = B // NCORES   # per-core batch
QW = NCH * BL      # Q-chain tile width (96)
T_MAX = 1000.0
KAPPA = 400.0      # 1/SIGMA_INIT^2
EPS = 1e-5
TWO_PI = 6.283185307179586
# ln(1/(2*pi*i0e(400)))  [i0e(400) = 0.019953356281939987]
LNC = 2.076480848703078
# cos(sqrt(u)) on u in [0, pi^2] (|delta| folded to [0,pi]), minimax power
# basis c0..c5 (max err 1.75e-6 -> 7e-4 on log p0; tolerance is 2e-2)
COS_COEF = [0.9999982503105576, -0.4999925129381312, 0.0416590259231213,
            -0.0013857591185452258, 2.419643469550081e-05,
            -2.1969780329048054e-07]
# degree-8 Taylor-fit coefficients for the f32 fallback path (err 4e-14)
COS_COEF8 = [1.00000000e+00, -5.00000000e-01, 4.16666666e-02, -1.38888885e-03,
             2.48015646e-05, -2.75566515e-07, 2.08651966e-09, -1.13535474e-11,
             4.13131734e-14]

_COMPILED = {}

# fast-path dtype mode: "bf16" (2-byte tiles, PE 1cyc/row, DVE 2x modes) or
# "f32r" (f32 tiles, matmul operands viewed as float32r: PE 1cyc/row on wide
# matmuls at full f32 storage precision).  Chosen empirically on hardware.
FAST_MODE = "bf16"


def _taylor_deg(x, tol, lo):
    """Smallest d with x^(d+1)/(d+1)! < tol."""
    d = lo
    term = x ** (d + 1) / math.factorial(d + 1)
    while term > tol and d < 40:
        d += 1
        term *= x / (d + 1)
    return d


def _plan(anorm):
    """Choose (k_bits, deg_p, deg_r, mode) from ||A||_inf.  The time grid is
    T0 = T_MAX/2^k_bits; every squaring level applies one bit of the
    quantized delay."""
    xa = T_MAX * float(anorm)
    if xa <= 0.0:
        return 3, 4, 3, FAST_MODE

    def pick(c_lvl, c_pre, c_tay, tol_r, tol_p_num):
        k0 = max(3, min(16, math.ceil(math.log2(max(xa / 0.9, 2.0)))))
        best = None
        for k in range(max(3, k0 - 1), min(16, k0 + 2) + 1):
            x0 = xa / (1 << k)
            tol_p = min(max(tol_p_num / 2 ** (k / 2), 5e-8), 2e-5)
            dp = _taylor_deg(x0, tol_p, 4)
            dr = _taylor_deg(x0, tol_r, 3)
            cost = (k - 1) * c_lvl + (dp - 1) * c_pre + dr * c_tay
            if best is None or cost < best[0]:
                best = (cost, k, dp, dr)
        return best[1], best[2], best[3]

    fast_c = {"bf16": (2.2, 1.6, 0.6), "f32r": (2.5, 1.7, 0.7)}[FAST_MODE]
    k, dp, dr = pick(*fast_c, 2e-4, 3e-4)
    if k <= 10:
        return k, dp, dr, FAST_MODE
    k, dp, dr = pick(7.6, 4.5, 0.7, 1e-6, 3e-5)
    return k, dp, dr, "f32"


def _build_bass(k_bits, deg_p, deg_r, mode):
    """Construct the Bass program (SPMD; identical on all 8 cores)."""
    import concourse.tile as tile
    from concourse import bacc, mybir

    F32 = mybir.dt.float32
    R32 = mybir.dt.float32r
    BF = mybir.dt.bfloat16 if mode == "bf16" else F32
    MDT = BF if mode == "bf16" else mybir.dt.uint8   # mask dtype
    AF = mybir.ActivationFunctionType
    OP = mybir.AluOpType
    cos_coef = COS_COEF if mode != "f32" else COS_COEF8

    def mv(ap):
        # matmul operand view: f32r reinterpretation in f32r mode
        return ap.bitcast(R32) if mode == "f32r" else ap

    nc = bacc.Bacc("TRN2", target_bir_lowering=False, debug=False)

    def din(name, shape, dt=F32):
        return nc.dram_tensor(name, shape, dt, kind="ExternalInput").ap()

    d_xx = din("xx", [P, 5 * N + 3 * P], BF)   # packed X / X^T pieces
    d_cmir = din("cmir", [P, NCH + BL])        # [c_mesh chunks | init rep]
    d_qtab = din("qtab", [P, (deg_r + 1) * QW], BF)  # [rdk | one-hot]
    d_msk = din("msk", [P, k_bits * QW], MDT)  # bit masks (0/1), x3 chunks
    d_out = nc.dram_tensor("terms", [1, BL], F32, kind="ExternalOutput").ap()

    with tile.TileContext(nc) as tc:
        with (
            tc.tile_pool(name="const", bufs=1) as cpool,
            tc.tile_pool(name="mats", bufs=3) as mpool,
            tc.tile_pool(name="qp", bufs=3) as qpool,
            tc.tile_pool(name="tp", bufs=4) as tpool,
            tc.tile_pool(name="psb", bufs=3, space="PSUM") as psb,
            tc.tile_pool(name="pst", bufs=3, space="PSUM") as pstp,
            tc.tile_pool(name="pss", bufs=2, space="PSUM") as pss,
        ):
            # ---- input DMAs: few, packed, in consumption order ------------
            # xx layout: [XT00 | XNr0 | XTr1 | XNr1 | XTr2 | XNr2 | XT0rest]
            # so each DMA lands exactly what the next prelude matmuls need.
            XXW = 5 * N + 3 * P
            XX = cpool.tile([P, XXW], BF, tag="xx")
            CMIR = cpool.tile([P, NCH + BL], F32, tag="cmir")
            QTAB = cpool.tile([P, (deg_r + 1) * QW], BF, tag="qtab")
            MSK = cpool.tile([P, k_bits * QW], MDT, tag="msk")
            cuts = [0, P + N, P + 3 * N, P + 5 * N, XXW]
            for a, b in zip(cuts[:-1], cuts[1:]):
                nc.sync.dma_start(XX[:, a:b], d_xx[:, a:b])
            nc.sync.dma_start(CMIR[:], d_cmir[:])
            nc.sync.dma_start(QTAB[:], d_qtab[:])
            nc.sync.dma_start(MSK[:], d_msk[:])
            CM = CMIR[:, 0:NCH]
            IREP = CMIR[:, NCH:NCH + BL]
            RDK = QTAB[:, 0:deg_r * QW]
            OH = QTAB[:, deg_r * QW:(deg_r + 1) * QW]

            def xn_s(c):
                # XN row-chunk c (rhs of prelude matmuls), contiguous
                o = P + 2 * c * N
                return XX[:, o:o + N]

            def xt_s(c, i):
                # XT block (row-chunk c, piece i) for prelude/taylor lhsT
                if c == 0:
                    o = 0 if i == 0 else P + 5 * N + (i - 1) * P
                else:
                    o = P + (2 * c - 1) * N + i * P
                return XX[:, o:o + P]

            ONES = cpool.tile([P, 1], BF, tag="ones")
            nc.vector.memset(ONES[:], 1.0)
            BEXP = cpool.tile([P, 1], F32, tag="bexp")
            nc.vector.memset(BEXP[:], LNC - KAPPA)
            BLN0 = cpool.tile([1, 1], F32, tag="bln0")
            nc.vector.memset(BLN0[:], 0.0)
            # dummy Ln first so the act-table pass loads the one set that
            # holds ln+exp+abs+copy up front (no mid-kernel table switch)
            LDUM = cpool.tile([1, 1], F32, tag="ldum")
            nc.scalar.activation(LDUM[:], BLN0[:], AF.Ln, bias=BLN0[:],
                                 scale=1.0)
            # identities built on device (no DMA)
            EYE = cpool.tile([P, NCH * N], BF, tag="eye")
            nc.vector.memset(EYE[:], 1.0)
            nc.gpsimd.affine_select(
                EYE[:].rearrange("p (c n) -> p c n", c=NCH),
                EYE[:].rearrange("p (c n) -> p c n", c=NCH),
                pattern=[[-P, NCH], [1, N]], compare_op=OP.is_equal,
                fill=0.0, base=0, channel_multiplier=-1,
            )
            E120 = cpool.tile([P, P], BF, tag="e120")
            nc.vector.memset(E120[:], 1.0)
            nc.gpsimd.affine_select(
                E120[:], E120[:], pattern=[[1, P]], compare_op=OP.is_equal,
                fill=0.0, base=0, channel_multiplier=-1,
            )

            def mm_group(ps, lhsT_of, rhs_of, i):
                # lhsT_of(c, i) -> [P,P] slice; rhs_of(c) -> [P,w] slice
                for c in range(NCH):
                    nc.tensor.matmul(
                        ps[:], lhsT=mv(lhsT_of(c, i)), rhs=mv(rhs_of(c)),
                        start=(c == 0), stop=(c == NCH - 1),
                    )

            def tile_b(tile_, c, i):
                return tile_[:, c * N + i * P: c * N + i * P + P]

            # ---- p0 (von Mises) in Q-layout [P, 3*BL] ---------------------
            Q0 = qpool.tile([P, QW], BF, tag="q")
            deg_c = len(cos_coef) - 1
            for c in range(NCH):
                qs = Q0[:, c * BL:(c + 1) * BL]
                dl = tpool.tile([P, BL], F32, tag="t0")
                # delta = init - c_mesh  (cos is even, sign irrelevant)
                nc.vector.tensor_scalar(dl[:], IREP[:], CM[:, c:c + 1], None,
                                        op0=OP.subtract)
                ab = tpool.tile([P, BL], F32, tag="t1")
                nc.scalar.activation(ab[:], dl[:], AF.Abs)
                fl = tpool.tile([P, BL], F32, tag="t2")
                nc.vector.tensor_scalar(fl[:], ab[:], -1.0, TWO_PI,
                                        op0=OP.mult, op1=OP.add)
                w = tpool.tile([P, BL], F32, tag="t3")
                nc.vector.tensor_tensor(w[:], ab[:], fl[:], op=OP.min)
                u = tpool.tile([P, BL], F32, tag="t0")
                nc.vector.tensor_tensor(u[:], w[:], w[:], op=OP.mult)
                h = tpool.tile([P, BL], F32, tag="t1")
                nc.vector.tensor_scalar(h[:], u[:], cos_coef[deg_c],
                                        cos_coef[deg_c - 1],
                                        op0=OP.mult, op1=OP.add)
                heng = nc.gpsimd if c == 1 else nc.vector
                for k in range(deg_c - 2, -1, -1):
                    heng.tensor_tensor(h[:], h[:], u[:], op=OP.mult)
                    heng.tensor_scalar(h[:], h[:], cos_coef[k], None,
                                       op0=OP.add)
                # p0 = exp(kappa*cos - kappa + lnC)
                nc.scalar.activation(qs, h[:], AF.Exp, bias=BEXP[:],
                                     scale=KAPPA)

            # ---- Q-chain step emitters (off the M-chain critical path) ----
            # residual Taylor: V <- Q0 + rdk_k*(X V), k=deg_r..1
            taylor_state = {"V": Q0, "k": deg_r}

            def taylor_step():
                k = taylor_state["k"]
                if k < 1:
                    return
                Vc = taylor_state["V"]
                ps = pss.tile([P, QW], F32, tag="ap")
                for i in range(NCH):
                    for c in range(NCH):
                        nc.tensor.matmul(
                            ps[:, i * BL:(i + 1) * BL],
                            lhsT=mv(xt_s(c, i)),
                            rhs=mv(Vc[:, c * BL:(c + 1) * BL]),
                            start=(c == 0), stop=(c == NCH - 1),
                        )
                Vn = qpool.tile([P, QW], BF, tag="v")
                nc.vector.tensor_tensor(Vn[:], ps[:],
                                        RDK[:, (k - 1) * QW: k * QW],
                                        op=OP.mult)
                nc.gpsimd.tensor_tensor(Vn[:], Vn[:], Q0[:], op=OP.add)
                taylor_state["V"] = Vn
                taylor_state["k"] = k - 1

            def apply_bit(MTj, q_tile, bit, blend_src=None):
                # Qn = bit ? M_j @ q : blend_src   (9 narrow mms + one blend)
                # the pass-through copy goes first: it only needs q, so it
                # overlaps the matmuls instead of serializing after them
                Qn = qpool.tile([P, QW], BF, tag="q")
                nc.gpsimd.tensor_copy(
                    Qn[:], (q_tile if blend_src is None else blend_src)[:])
                ps = pss.tile([P, QW], F32, tag="ap")
                for i in range(NCH):
                    for c in range(NCH):
                        nc.tensor.matmul(
                            ps[:, i * BL:(i + 1) * BL],
                            lhsT=mv(tile_b(MTj, c, i)),
                            rhs=mv(q_tile[:, c * BL:(c + 1) * BL]),
                            start=(c == 0), stop=(c == NCH - 1),
                        )
                nc.vector.copy_predicated(
                    Qn[:], MSK[:, bit * QW:(bit + 1) * QW], ps[:])
                return Qn

            # ---- prelude: ascending Taylor S = I + sum X^k/k! -------------
            S = mpool.tile([P, NCH * N], BF, tag="M")
            for c in range(NCH):
                nc.vector.tensor_tensor(S[:, c * N:(c + 1) * N], xn_s(c),
                                        EYE[:, c * N:(c + 1) * N], op=OP.add)
            T = None   # None -> XN accessor
            for k in range(2, deg_p + 1):
                Tn = mpool.tile([P, NCH * N], BF, tag="T")
                rhs_of = xn_s if T is None else (
                    lambda c, _T=T: _T[:, c * N:(c + 1) * N])
                for i in range(NCH):
                    ps = psb.tile([P, N], F32, tag="sq")
                    mm_group(ps, xt_s, rhs_of, i)
                    # scale-evac feeds the next step's matmuls; the S
                    # accumulation runs on DVE off the PE path
                    dst = Tn[:, i * N:(i + 1) * N]
                    if i == 1:
                        nc.gpsimd.tensor_scalar(dst, ps[:], 1.0 / k, None,
                                                op0=OP.mult)
                    else:
                        nc.scalar.mul(dst, ps[:], 1.0 / k)
                    nc.vector.tensor_tensor(S[:, i * N:(i + 1) * N],
                                            S[:, i * N:(i + 1) * N], dst,
                                            op=OP.add)
                T = Tn
                taylor_step()
            while taylor_state["k"] >= 1:   # drain (off critical path)
                taylor_step()
            QB = taylor_state["V"]

            ev = {"i": 0}

            def piece_evac(dst, src):
                e = ev["i"] % 3
                ev["i"] += 1
                if e == 0:
                    nc.vector.tensor_copy(dst, src)
                elif e == 1:
                    nc.scalar.copy(dst, src)
                else:
                    nc.gpsimd.tensor_copy(dst, src)

            def transpose_mq(MTt, Mt):
                # transposes ordered by source S-row (ib) so each trio only
                # waits its own row's evacuation; every [P,P] block
                # evacuates immediately so the next level's first matmuls
                # (which need MT row c piece 0 = S blocks (0,c)) unblock as
                # early as possible.
                for ib in range(NCH):
                    pt = pstp.tile([P, N], BF, tag="tr")
                    for cp in range(NCH):
                        nc.tensor.transpose(
                            mv(pt[:, cp * P:(cp + 1) * P]),
                            mv(Mt[:, ib * N + cp * P: ib * N + cp * P + P]),
                            mv(E120[:]),
                        )
                        piece_evac(MTt[:, cp * N + ib * P: cp * N + ib * P + P],
                                   pt[:, cp * P:(cp + 1) * P])

            MT = mpool.tile([P, NCH * N], BF, tag="MT")
            transpose_mq(MT, S)

            # ---- chain: squarings with lag-1 merged bit applies -----------
            # squaring s: M_s = M_{s-1}^2 (wide mms + transposes); bit s-1
            # applied right after the wide mms (its operands are a full level
            # old, so the PE never stalls on the Q-chain).
            n_sq = k_bits - 2
            for s in range(1, n_sq + 1):
                last = s == n_sq
                if not last:
                    Sn = mpool.tile([P, NCH * N], BF, tag="M")
                    for i in range(NCH):
                        ps = psb.tile([P, N], F32, tag="sq")
                        mm_group(ps, lambda c, ii, _M=MT: tile_b(_M, c, ii),
                                 lambda c, _S=S: _S[:, c * N:(c + 1) * N], i)
                        dst = Sn[:, i * N:(i + 1) * N]
                        if i == 0:
                            nc.gpsimd.tensor_copy(dst, ps[:])
                        elif i == 1:
                            nc.scalar.copy(dst, ps[:])
                        else:
                            # the last row gates this level's transposes:
                            # split it across DVE+ACT so it lands fastest
                            h = N // 2
                            nc.vector.tensor_copy(dst[:, 0:h], ps[:, 0:h])
                            nc.scalar.copy(dst[:, h:N], ps[:, h:N])
                else:
                    # last squaring: only M_{n_sq}^T is ever consumed (by the
                    # top-bit applies) -> compute S^T = M^T@M^T directly and
                    # skip the row form + transposes entirely
                    MTn = mpool.tile([P, NCH * N], BF, tag="MT")
                    for i in range(NCH):
                        ps = psb.tile([P, N], F32, tag="sq")
                        for c in range(NCH):
                            nc.tensor.matmul(
                                ps[:], lhsT=mv(tile_b(S, c, i)),
                                rhs=mv(MT[:, c * N: (c + 1) * N]),
                                start=(c == 0), stop=(c == NCH - 1),
                            )
                        eng = (nc.gpsimd, nc.scalar, nc.vector)[i]
                        if eng is nc.scalar:
                            nc.scalar.copy(MTn[:, i * N:(i + 1) * N], ps[:])
                        elif eng is nc.vector:
                            nc.vector.tensor_copy(MTn[:, i * N:(i + 1) * N],
                                                  ps[:])
                        else:
                            nc.gpsimd.tensor_copy(MTn[:, i * N:(i + 1) * N],
                                                  ps[:])
                # bit s-1 rides on M_{s-1} (= current MT) while evacs run
                QB = apply_bit(MT, QB, s - 1)
                if not last:
                    MTn = mpool.tile([P, NCH * N], BF, tag="MT")
                    transpose_mq(MTn, Sn)
                    S = Sn
                MT = MTn

            # ---- top two bits: single + double apply of M_{k-2} -----------
            QB = apply_bit(MT, QB, k_bits - 2)
            psy = pss.tile([P, QW], F32, tag="ap")
            for i in range(NCH):
                for c in range(NCH):
                    nc.tensor.matmul(
                        psy[:, i * BL:(i + 1) * BL],
                        lhsT=mv(tile_b(MT, c, i)),
                        rhs=mv(QB[:, c * BL:(c + 1) * BL]),
                        start=(c == 0), stop=(c == NCH - 1),
                    )
            Y1 = qpool.tile([P, QW], BF, tag="v")
            nc.vector.tensor_copy(Y1[:], psy[:])
            Vf = apply_bit(MT, Y1, k_bits - 1, blend_src=QB)

            # ---- selection + loss terms -----------------------------------
            sel = psb.tile([1, BL], F32, tag="sq")
            tmp = tpool.tile([P, QW], BF, tag="sel")
            nc.vector.tensor_tensor(tmp[:], Vf[:], OH[:], op=OP.mult)
            for c in range(NCH):
                nc.tensor.matmul(sel[:], lhsT=mv(ONES[:]),
                                 rhs=mv(tmp[:, c * BL:(c + 1) * BL]),
                                 start=(c == 0), stop=(c == NCH - 1))
            # ln(relu(psel)+eps) via exponent/mantissa split: the HW Ln
            # table degrades for huge args (psel can reach ~1e20 in the
            # weak-diffusion regime), so compute ln(m) + e*ln2 with m in
            # [1,2), which keeps the table in its accurate range.
            I32 = mybir.dt.int32
            rl = tpool.tile([1, BL], F32, tag="r0")
            nc.vector.tensor_scalar(rl[:], sel[:], 0.0, EPS,
                                    op0=OP.max, op1=OP.add)
            xi = rl[:].bitcast(I32)
            et = tpool.tile([1, BL], I32, tag="r2")
            nc.vector.tensor_scalar(et[:], xi, 23, None,
                                    op0=OP.arith_shift_right)
            ef = tpool.tile([1, BL], F32, tag="r3")
            nc.vector.tensor_copy(ef[:], et[:])
            mi = tpool.tile([1, BL], I32, tag="r4")
            nc.vector.tensor_scalar(mi[:], xi, 0x007FFFFF, 0x3F800000,
                                    op0=OP.bitwise_and, op1=OP.bitwise_or)
            lnm = tpool.tile([1, BL], F32, tag="r5")
            nc.scalar.activation(lnm[:], mi[:].bitcast(F32), AF.Ln,
                                 bias=BLN0[:], scale=1.0)
            terms = tpool.tile([1, BL], F32, tag="r1")
            # ef holds the biased exponent; fold the -127*ln2 into the mult
            nc.vector.tensor_scalar(terms[:], ef[:], 0.6931471805599453,
                                    -88.02969193111305,
                                    op0=OP.mult, op1=OP.add)
            nc.vector.tensor_tensor(terms[:], terms[:], lnm[:], op=OP.add)
            nc.sync.dma_start(d_out[:], terms[:])

    nc.compile()
    return nc


def _host_prep(c_mesh, gtheta, sigma_diff, init_color, delay_t, report_color):
    """Host-side glue: operator assembly (replicating reference f32 ops),
    plan selection, and per-core index/bit/layout arrays."""
    import ml_dtypes
    f32 = np.float32
    c = np.asarray(c_mesh, dtype=f32)
    g = np.asarray(gtheta, dtype=f32)
    s = np.asarray(sigma_diff, dtype=f32)[0]
    init = np.asarray(init_color, dtype=f32)
    t = np.asarray(delay_t, dtype=f32)
    rep = np.asarray(report_color, dtype=f32)

    d = (c[1] - c[0]).astype(f32)
    eye = np.eye(N, dtype=f32)
    up = np.roll(eye, -1, axis=1)
    dn = np.roll(eye, 1, axis=1)
    D1 = ((up - dn) / (f32(2.0) * d)).astype(f32)
    D2 = ((up - f32(2.0) * eye + dn) / (d * d)).astype(f32)
    A = ((s ** f32(2.0)) / f32(2.0) * D2 - D1 * g[None, :]).astype(f32)

    anorm = np.abs(A.astype(np.float64)).sum(axis=1).max()
    k_bits, deg_p, deg_r, mode = plan = _plan(anorm)
    bf = ml_dtypes.bfloat16 if mode == "bf16" else f32
    mdt = ml_dtypes.bfloat16 if mode == "bf16" else np.uint8
    T0 = T_MAX / (1 << k_bits)
    X = (A * f32(T0)).astype(f32)

    m = np.floor(t.astype(np.float64) / T0).astype(np.int64)
    m = np.clip(m, 0, (1 << k_bits) - 1)
    r = (t.astype(np.float64) - m * T0) / T0  # in X = T0*A units
    bits = ((m[:, None] >> np.arange(k_bits)[None, :]) & 1)     # [B, K]
    idx = np.argmin(np.abs(c[None, :] - rep[:, None]), axis=1)

    # packed matrix buffer in DMA/consumption order:
    # [XT00 | XNr0 | XTr1 | XNr1 | XTr2 | XNr2 | XT0rest]
    XT_ = np.ascontiguousarray(X.T)
    xx = np.concatenate([
        XT_[0:P, 0:P],
        X[0:P, :], XT_[P:2 * P, :],
        X[P:2 * P, :], XT_[2 * P:3 * P, :],
        X[2 * P:3 * P, :], XT_[0:P, P:N],
    ], axis=1)
    cm = np.ascontiguousarray(c.reshape(NCH, P).T)
    shared = {"xx": xx.astype(bf)}
    in_maps = []
    for core in range(NCORES):
        sl = slice(core * BL, (core + 1) * BL)
        irep = np.broadcast_to(init[sl][None, :], (P, BL)).astype(f32)
        cmir = np.concatenate([cm, irep], axis=1).astype(f32)
        # bit j replicated x3 (one copy per Q chunk) at [j*QW:(j+1)*QW]
        msk = np.broadcast_to(
            np.tile(bits[sl].T.reshape(k_bits, 1, BL), (1, NCH, 1))
            .reshape(1, k_bits * QW), (P, k_bits * QW)).astype(mdt)
        rdk = np.empty((deg_r, BL), f32)
        for k in range(1, deg_r + 1):
            rdk[k - 1] = (r[sl] / k).astype(f32)
        rdk = np.tile(rdk.reshape(deg_r, 1, BL), (1, NCH, 1)).reshape(
            1, deg_r * QW)
        oh = np.zeros((NCH, P, BL), f32)
        for b, ix in enumerate(idx[sl]):
            oh[ix // P, ix % P, b] = 1.0
        oh = oh.transpose(1, 0, 2).reshape(P, QW)
        qtab = np.concatenate(
            [np.broadcast_to(rdk, (P, deg_r * QW)), oh], axis=1).astype(bf)
        in_maps.append(dict(shared, cmir=cmir, msk=np.ascontiguousarray(msk),
                            qtab=np.ascontiguousarray(qtab)))
    return plan, in_maps


def _get_nc(plan):
    if plan not in _COMPILED:
        _COMPILED[plan] = _build_bass(*plan)
    return _COMPILED[plan]


def kernel(**inputs):
    from concourse.bass_utils import run_bass_kernel_spmd

    plan, in_maps = _host_prep(
        inputs["c_mesh"], inputs["gtheta"], inputs["sigma_diff"],
        inputs["init_color"], inputs["delay_t"], inputs["report_color"],
    )
    nc = _get_nc(plan)
    res = run_bass_kernel_spmd(nc, in_maps, list(range(NCORES)))
    terms = np.concatenate(
        [np.asarray(res.results[k]["terms"]).reshape(-1) for k in range(NCORES)]
    )
    loss = -np.mean(terms.astype(np.float64))
    return np.asarray(loss, dtype=np.float32)
